# revision 5
# baseline (speedup 1.0000x reference)
"""KgAdapterCrossAttention kernel for 8 trn2 NeuronCores — v2.

Sharding: core = (batch b, query-half qh). Each core: 1024 queries x 2048 keys,
4 heads of 64 dims.

Techniques vs baseline (fp32 everywhere):
  - projections and A/O matmuls in bf16 (PE 1 cycle/row vs fp32's 4),
  - scores via error-compensated fp8e4m3 DoubleRow: with q = q_hi + q_lo and
    k = k_hi + k_lo (residual quantization), two DR matmuls per (head, ktile)
    compute K_hi'(q_hi + q_lo) and K_lo'q_hi + I*logM at 0.5 cycles/row —
    near-full-precision scores at half the bf16 PE cost, with the align-mask
    add fused into the last DR slot for free,
  - exp split across three engines: Act (native Exp -> bf16 P) plus DVE and
    Pool (Schraudolph fast-exp: int16 bits = S*184.665 + B, bitcast to bf16;
    the fused logM (-30) makes masked entries ~2^-40),
  - per-head softmax denominator from a ones-column appended to V,
  - O projection in bf16 after PE transposes.

Softmax has no max-subtraction; all logits carry a uniform -4 shift (softmax
invariant) so exp stays small; Schraudolph's constant factor cancels in the
p/sum(p) ratio.
"""

import os
import sys

import numpy as np

try:
    import concourse.bass as bass
except ImportError:
    for _p in ("/opt/trn_rl_repo", os.path.expanduser("~/.axon_site/_ro/trn_rl_repo")):
        if os.path.isdir(_p) and _p not in sys.path:
            sys.path.insert(0, _p)
    import concourse.bass as bass

import ml_dtypes
import concourse.mybir as mybir
import concourse.tile as tile
from concourse import bacc
from concourse.masks import make_identity
from contextlib import ExitStack

F32 = mybir.dt.float32
BF16 = mybir.dt.bfloat16
FP8 = mybir.dt.float8e4
I16 = mybir.dt.int16
EXP = mybir.ActivationFunctionType.Exp
DR = mybir.MatmulPerfMode.DoubleRow
MUL = mybir.AluOpType.mult
ADD = mybir.AluOpType.add
SUB = mybir.AluOpType.subtract

P = 128
HID = 256
NHEAD = 4
DHEAD = 64
NQ = 1024  # queries per core
NK = 2048  # keys (full)
QBLK = 256
NQB = NQ // QBLK  # 4
NKT = NK // P  # 16
NCT = HID // P  # 2

# ktile assignment to exp engines: Act | DVE | Pool
NA = 10
ND = 6
NP = NKT - NA - ND  # 0 (gpsimd cannot access PSUM)

LOGM = -30.0   # masked-out additive bias (exact in e4m3); exp(-30+s) ~ 0
SHIFT = -4.0   # uniform logit shift (softmax-invariant): max score ~8.2 stays accurate
SCH_A = 184.664965  # 2^7 / ln 2
SCH_B = 16253.0     # 127*2^7 - 3 (Schraudolph bias; constant factor cancels)

# scr free-layout (fp8 elements per partition):
#   q region: (t, par) x (hi, lo) -> 8 x 1024, then logM double buffer 2 x 4096
SCR_Q = 8 * NQ            # 8192
SCR_W = SCR_Q + 2 * NKT * QBLK  # 16384
# kt8 free-layout per t-half: [ K_hi 2048 | K_lo 2048 | identity 128 ]
KT8_W = 2 * NK + P  # 4224


def _q_off(t: int, par: int, lo: int, qb: int) -> int:
    return ((2 * t + par) * 2 + lo) * NQ + qb * QBLK


def _m_off(qb: int, kt: int) -> int:
    return SCR_Q + (qb % 2) * NKT * QBLK + kt * QBLK


def build(with_attn_mask: bool) -> bass.Bass:
    del with_attn_mask  # attn mask folded into logm8 on host
    nc = bacc.Bacc()
    xqT = nc.declare_dram_parameter("xqT", [HID, NQ], BF16, isOutput=False)
    xkT = nc.declare_dram_parameter("xkT", [HID, NK], BF16, isOutput=False)
    wqT = nc.declare_dram_parameter("wqT", [HID, HID], BF16, isOutput=False)
    wkT = nc.declare_dram_parameter("wkT", [HID, HID], BF16, isOutput=False)
    wvT = nc.declare_dram_parameter("wvT", [HID, HID], BF16, isOutput=False)
    woT = nc.declare_dram_parameter("woT", [HID, HID], BF16, isOutput=False)
    logm8 = nc.declare_dram_parameter("logm8", [NQB, P, NKT, QBLK], FP8, isOutput=False)
    out_d = nc.declare_dram_parameter("out", [NQ, HID], F32, isOutput=True)

    with tile.TileContext(nc) as tc, ExitStack() as ctx:
        const = ctx.enter_context(tc.tile_pool(name="const", bufs=1))
        big = ctx.enter_context(tc.tile_pool(name="big", bufs=1))
        p16p = ctx.enter_context(tc.tile_pool(name="p16p", bufs=2))
        anp = ctx.enter_context(tc.tile_pool(name="anp", bufs=2))
        wrk = ctx.enter_context(tc.tile_pool(name="wrk", bufs=4))
        outp = ctx.enter_context(tc.tile_pool(name="outp", bufs=3))

        # --- weights + activations ---
        def load2(name, src, width):
            ts = []
            for t in range(2):
                tl = const.tile([P, width], BF16, tag=f"{name}{t}", name=f"{name}{t}")
                nc.sync.dma_start(out=tl, in_=src[t * P : (t + 1) * P, :])
                ts.append(tl)
            return ts

        wq_sb = load2("wq", wqT, HID)
        xq_sb = [big.tile([P, NQ], BF16, tag=f"xq{t}", name=f"xq{t}") for t in range(2)]
        xk_sb = [big.tile([P, NK], BF16, tag=f"xk{t}", name=f"xk{t}") for t in range(2)]
        for nb in range(NQ // 512):
            for t in range(2):
                nc.sync.dma_start(
                    out=xq_sb[t][:, nb * 512 : (nb + 1) * 512],
                    in_=xqT[t * P : (t + 1) * P, nb * 512 : (nb + 1) * 512],
                )
        wk_sb = load2("wk", wkT, HID)
        for nb in range(NK // 512):
            for t in range(2):
                nc.sync.dma_start(
                    out=xk_sb[t][:, nb * 512 : (nb + 1) * 512],
                    in_=xkT[t * P : (t + 1) * P, nb * 512 : (nb + 1) * 512],
                )
        wv_sb = load2("wv", wvT, HID)
        wo_sb = load2("wo", woT, HID)

        ident16 = const.tile([P, P], BF16, tag="id16", name="id16")
        make_identity(nc, ident16)

        # fused scores scratch: q hi/lo halves (zero-padded) + logM double buffer
        scr = big.tile([P, SCR_W], FP8, tag="scr", name="scr")
        # kt8 per t-half: [K_hi | K_lo | identity] fp8
        kt8 = [big.tile([P, KT8_W], FP8, tag=f"kt8_{t}", name=f"kt8_{t}") for t in range(2)]
        for t in range(2):
            idap = kt8[t][:, 2 * NK : 2 * NK + P]
            nc.gpsimd.memset(idap, 0.0)
            nc.gpsimd.affine_select(
                out=idap, in_=idap, compare_op=mybir.AluOpType.not_equal,
                fill=32.0, base=0, pattern=[[-1, P]], channel_multiplier=1,
            )

        # zero the dead half-rows of q regions (even heads live in rows 0:64,
        # odd heads in rows 64:128; the other half must be 0 for the fused DR)
        for t in range(2):
            for lo in range(2):
                o0 = _q_off(t, 0, lo, 0)
                o1 = _q_off(t, 1, lo, 0)
                nc.scalar.memzero(scr[DHEAD:P, o0 : o0 + NQ])
                nc.scalar.memzero(scr[0:DHEAD, o1 : o1 + NQ])

        # V (bf16) with denominator ones-column
        v16 = big.tile([P, NKT, NHEAD, DHEAD + 1], BF16, tag="v16", name="v16")
        nc.gpsimd.memset(v16[:, :, :, DHEAD : DHEAD + 1], 1.0)

        with ExitStack() as pctx:
            ps_pr = pctx.enter_context(tc.tile_pool(name="ps_pr", bufs=8, space="PSUM"))
            # Q projection -> q_hi fp8 + q_lo fp8 (head-parity split, zero halves)
            for t in range(2):
                for nb in range(NQ // 512):
                    ps = ps_pr.tile([P, 512], F32, tag="pr", name="pr")
                    for ct in range(NCT):
                        nc.tensor.matmul(
                            ps,
                            lhsT=wq_sb[ct][:, t * P : (t + 1) * P],
                            rhs=xq_sb[ct][:, nb * 512 : (nb + 1) * 512],
                            start=(ct == 0),
                            stop=(ct == NCT - 1),
                        )
                    for par, rows in ((0, slice(0, DHEAD)), (1, slice(DHEAD, P))):
                        hi = _q_off(t, par, 0, 0) + nb * 512
                        lo = _q_off(t, par, 1, 0) + nb * 512
                        nc.vector.tensor_copy(scr[rows, hi : hi + 512], ps[rows, :])
                        nc.vector.scalar_tensor_tensor(
                            scr[rows, lo : lo + 512],
                            ps[rows, :],
                            1.0,
                            scr[rows, hi : hi + 512],
                            op0=MUL,
                            op1=SUB,
                        )
            # K projection -> k_hi + k_lo fp8
            for t in range(2):
                for nb in range(NK // 512):
                    ps = ps_pr.tile([P, 512], F32, tag="pr", name="pr")
                    for ct in range(NCT):
                        nc.tensor.matmul(
                            ps,
                            lhsT=wk_sb[ct][:, t * P : (t + 1) * P],
                            rhs=xk_sb[ct][:, nb * 512 : (nb + 1) * 512],
                            start=(ct == 0),
                            stop=(ct == NCT - 1),
                        )
                    c0 = nb * 512
                    nc.scalar.copy(kt8[t][:, c0 : c0 + 512], ps)
                    nc.vector.scalar_tensor_tensor(
                        kt8[t][:, NK + c0 : NK + c0 + 512],
                        ps,
                        1.0,
                        kt8[t][:, c0 : c0 + 512],
                        op0=MUL,
                        op1=SUB,
                    )
            # V projection -> v16 bf16
            for kt in range(NKT):
                ps = ps_pr.tile([P, HID], F32, tag="pr", name="pr")
                for ct in range(NCT):
                    nc.tensor.matmul(
                        ps,
                        lhsT=xk_sb[ct][:, kt * P : (kt + 1) * P],
                        rhs=wv_sb[ct],
                        start=(ct == 0),
                        stop=(ct == NCT - 1),
                    )
                psr = ps.rearrange("p (h d) -> p h d", h=NHEAD)
                if kt % 2 == 0:
                    nc.vector.tensor_copy(v16[:, kt, :, 0:DHEAD], psr)
                else:
                    nc.scalar.copy(v16[:, kt, :, 0:DHEAD], psr)

        # --- attention ---
        with ExitStack() as actx:
            ps_sc = actx.enter_context(tc.tile_pool(name="ps_sc", bufs=3, space="PSUM"))
            ps_ao = actx.enter_context(tc.tile_pool(name="ps_ao", bufs=2, space="PSUM"))

            GROUPS = [list(range(g * 4, (g + 1) * 4)) for g in range(NKT // 4)]

            def seg_engine(kt):
                return 0 if kt < NA else (1 if kt < NA + ND else 2)

            def emit_score_group(qb, h, p16, grp, seg_fn=None):
                """PE: 2 fused DR matmuls per ktile; Act/DVE/Pool drain to P."""
                seg = seg_fn or seg_engine
                t, par = h // 2, h % 2
                qhi = _q_off(t, par, 0, qb)
                qlo = _q_off(t, par, 1, qb)
                ps = ps_sc.tile([P, 4, QBLK], F32, tag="sc", name="sc")
                for j, kt in enumerate(grp):
                    # DR1: K_hi' q_hi + K_hi' q_lo
                    lhs1 = bass.AP(
                        kt8[t].tensor,
                        kt8[t].offset + kt * P,
                        [[KT8_W, P], [0, 2], [1, P]],
                    )
                    rhs1 = bass.AP(
                        scr.tensor,
                        scr.offset + qhi,
                        [[SCR_W, P], [qlo - qhi, 2], [1, QBLK]],
                    )
                    nc.tensor.matmul(
                        ps[:, j, :], lhsT=lhs1, rhs=rhs1,
                        start=True, stop=False, perf_mode=DR,
                    )
                    # DR2: K_lo' q_hi + I * logM
                    lhs2 = bass.AP(
                        kt8[t].tensor,
                        kt8[t].offset + NK + kt * P,
                        [[KT8_W, P], [NK - kt * P, 2], [1, P]],
                    )
                    rhs2 = bass.AP(
                        scr.tensor,
                        scr.offset + qhi,
                        [[SCR_W, P], [_m_off(qb, kt) - qhi, 2], [1, QBLK]],
                    )
                    nc.tensor.matmul(
                        ps[:, j, :], lhsT=lhs2, rhs=rhs2,
                        start=False, stop=True, perf_mode=DR,
                    )
                # drain by engine segment (kt ranges are contiguous)
                j0 = 0
                while j0 < 4:
                    eng = seg(grp[j0])
                    j1 = j0
                    while j1 < 4 and seg(grp[j1]) == eng:
                        j1 += 1
                    src = ps[:, j0:j1, :]
                    dst = p16[:, grp[j0] : grp[j1 - 1] + 1, :]
                    if eng == 0:
                        nc.scalar.activation(dst, src, EXP, scale=float(1.0 / SCH_A))
                    else:
                        nc.vector.tensor_scalar(
                            dst.bitcast(I16), src, SCH_B, None, ADD
                        )
                    j0 = j1

            def emit_a_qt(st, qt):
                """PE: A(qt) = P^T V (+denom col); DVE recip; Pool normalize."""
                qb, h, p16, anorm, ap_t = st
                ap_ = ap_t[:, qt, :]
                qsl = slice(qt * P, (qt + 1) * P)
                for kt in range(NKT):
                    nc.tensor.matmul(
                        ap_,
                        lhsT=p16[:, kt, qsl],
                        rhs=v16[:, kt, h, :],
                        start=(kt == 0),
                        stop=(kt == NKT - 1),
                    )
                rec = wrk.tile([P, 1], F32, tag="rec", name="rec")
                nc.vector.reciprocal(rec, ap_[:, DHEAD : DHEAD + 1])
                nc.vector.tensor_scalar(
                    anorm[:, qt, h * DHEAD : (h + 1) * DHEAD],
                    ap_[:, 0:DHEAD],
                    rec,
                    None,
                    MUL,
                )

            def emit_oproj_qt(qb, anorm, qt):
                # anorm^T via DMA transpose (no PE/DVE cost), then bf16 matmul;
                # o accum shares the ps_ao union bank with the A tile
                bank = ps_ao.tile([P, 512], F32, tag="ao", name="ao")
                o_ps = bank[:, 0:256]
                atts = []
                for ct in range(NCT):
                    att = wrk.tile([P, P], BF16, tag=f"att{ct}", name=f"att{ct}")
                    nc.sync.dma_start_transpose(
                        att, anorm[:, qt, ct * P : (ct + 1) * P]
                    )
                    atts.append(att)
                for ct in range(NCT):
                    nc.tensor.matmul(
                        o_ps, lhsT=atts[ct], rhs=wo_sb[ct],
                        start=(ct == 0), stop=(ct == NCT - 1),
                    )
                ob = outp.tile([P, HID], F32, tag="ob", name="ob")
                nc.vector.tensor_copy(ob, o_ps)
                q0 = qb * QBLK + qt * P
                nc.sync.dma_start(out=out_d[q0 : q0 + P, :], in_=ob)

            # Software pipeline: A(i) (two qt halves) and O-proj quarters are
            # interleaved between score groups of step i+1 so PE never waits
            # on exp drains and the clock ramp is preserved.
            steps = [(qb, h) for qb in range(NQB) for h in range(NHEAD)]
            pend_a = None  # (qb, h, p16, anorm, ap_t)
            pend_o = []    # [(qb, anorm, qt), ...]
            anorms = {}
            def fetch_logm(qb):
                mb = _m_off(qb, 0)
                nc.sync.dma_start(
                    out=scr[:, mb : mb + NKT * QBLK],
                    in_=logm8[qb].rearrange("p t q -> p (t q)"),
                )

            fetch_logm(0)
            for qb, h in steps:
                if h == 0:
                    anorms[qb] = anp.tile([P, 2, HID], BF16, tag="an", name="an")
                if h == 1 and qb + 1 < NQB:
                    fetch_logm(qb + 1)
                p16 = p16p.tile([P, NKT, QBLK], BF16, tag="p16", name="p16")
                last = qb == NQB - 1 and h == NHEAD - 1
                seg_fn = (lambda kt: (0, 1, 0, 1, 0, 1, 0, 1, 0, 1, 0, 1, 0, 1, 0, 1)[kt]) if last else None
                emit_score_group(qb, h, p16, GROUPS[0], seg_fn)
                emit_score_group(qb, h, p16, GROUPS[1], seg_fn)
                if pend_a is not None:
                    emit_a_qt(pend_a, 0)
                emit_score_group(qb, h, p16, GROUPS[2], seg_fn)
                if pend_a is not None:
                    emit_a_qt(pend_a, 1)
                emit_score_group(qb, h, p16, GROUPS[3], seg_fn)
                if pend_o:
                    emit_oproj_qt(*pend_o.pop(0))
                abank = ps_ao.tile([P, 512], F32, tag="ao", name="ao")
                ap_t = abank[:, 256 : 256 + 2 * (DHEAD + 1)].rearrange(
                    "p (a b) -> p a b", a=2
                )
                pend_a = (qb, h, p16, anorms[qb], ap_t)
                if h == 1 and qb > 0:
                    an = anorms.pop(qb - 1)
                    pend_o += [(qb - 1, an, 0), (qb - 1, an, 1)]
                if last:
                    # drain tail immediately, A interleaved per group already
                    emit_a_qt(pend_a, 0)
                    emit_a_qt(pend_a, 1)
                    pend_a = None
                    an = anorms.pop(NQB - 1)
                    emit_oproj_qt(NQB - 1, an, 0)
                    emit_oproj_qt(NQB - 1, an, 1)
    nc.compile()
    return nc


_NC_CACHE = {}
_last_in_maps = None


def _get_nc(with_attn_mask: bool = False) -> bass.Bass:
    if with_attn_mask not in _NC_CACHE:
        _NC_CACHE[with_attn_mask] = build(with_attn_mask)
    return _NC_CACHE[with_attn_mask]


def kernel(q_hidden_states, k_hidden_states, attention_mask, align_mask, Wq, Wk, Wv, Wo):
    from concourse.bass_utils import run_bass_kernel_spmd

    q_hidden_states = np.asarray(q_hidden_states, np.float32)
    k_hidden_states = np.asarray(k_hidden_states, np.float32)
    attention_mask = np.asarray(attention_mask, np.float32)
    align_mask = np.asarray(align_mask)
    B, Q, _ = q_hidden_states.shape
    qh_len = Q // 2  # 1024

    use_mask = bool(np.any(attention_mask))
    nc = _get_nc(False)

    bf16 = ml_dtypes.bfloat16
    f8 = ml_dtypes.float8_e4m3
    s8 = np.float32(np.sqrt(SCH_A) / np.sqrt(8.0))
    wq = np.ascontiguousarray(np.asarray(Wq, np.float32).T * s8).astype(bf16)
    wk = np.ascontiguousarray(np.asarray(Wk, np.float32).T * s8).astype(bf16)
    wv = np.ascontiguousarray(np.asarray(Wv, np.float32).T).astype(bf16)
    wo = np.ascontiguousarray(np.asarray(Wo, np.float32).T).astype(bf16)

    in_maps = []
    for core in range(8):
        b, qh = divmod(core, 2)
        qsl = slice(qh * qh_len, (qh + 1) * qh_len)
        logm = np.where(align_mask[b, :, qsl] == 0, np.float32(LOGM), np.float32(SHIFT))
        if use_mask:
            amk = attention_mask[b, 0, qsl, :].T  # [K, Q-slice]
            logm = np.maximum(logm + amk, np.float32(LOGM))
        logm = logm * np.float32(SCH_A / 32.0)
        # [K, Qh] -> [qb, p, t, q] with k = t*128 + p, qcol = qb*256 + q
        lm = logm.reshape(NKT, P, NQB, QBLK).transpose(2, 1, 0, 3)
        m = {
            "xqT": np.ascontiguousarray(q_hidden_states[b, qsl].T).astype(bf16),
            "xkT": np.ascontiguousarray(k_hidden_states[b].T).astype(bf16),
            "wqT": wq,
            "wkT": wk,
            "wvT": wv,
            "woT": wo,
            "logm8": np.ascontiguousarray(lm).astype(f8),
        }
        in_maps.append(m)

    global _last_in_maps
    _last_in_maps = in_maps
    res = run_bass_kernel_spmd(nc, in_maps, list(range(8))).results
    out = np.empty((B, Q, HID), np.float32)
    for core in range(8):
        b, qh = divmod(core, 2)
        out[b, qh * qh_len : (qh + 1) * qh_len] = res[core]["out"]
    return out


# revision 6
# speedup vs baseline: 1.0489x; 1.0489x over previous
"""KgAdapterCrossAttention kernel for 8 trn2 NeuronCores — v2.

Sharding: core = (batch b, query-half qh). Each core: 1024 queries x 2048 keys,
4 heads of 64 dims.

Techniques vs baseline (fp32 everywhere):
  - projections and A/O matmuls in bf16 (PE 1 cycle/row vs fp32's 4),
  - scores via error-compensated fp8e4m3 DoubleRow: with q = q_hi + q_lo and
    k = k_hi + k_lo (residual quantization), two DR matmuls per (head, ktile)
    compute K_hi'(q_hi + q_lo) and K_lo'q_hi + I*logM at 0.5 cycles/row —
    near-full-precision scores at half the bf16 PE cost, with the align-mask
    add fused into the last DR slot for free,
  - exp split across three engines: Act (native Exp -> bf16 P) plus DVE and
    Pool (Schraudolph fast-exp: int16 bits = S*184.665 + B, bitcast to bf16;
    the fused logM (-30) makes masked entries ~2^-40),
  - per-head softmax denominator from a ones-column appended to V,
  - O projection in bf16 after PE transposes.

Softmax has no max-subtraction; all logits carry a uniform -4 shift (softmax
invariant) so exp stays small; Schraudolph's constant factor cancels in the
p/sum(p) ratio.
"""

import os
import sys

import numpy as np

try:
    import concourse.bass as bass
except ImportError:
    for _p in ("/opt/trn_rl_repo", os.path.expanduser("~/.axon_site/_ro/trn_rl_repo")):
        if os.path.isdir(_p) and _p not in sys.path:
            sys.path.insert(0, _p)
    import concourse.bass as bass

import ml_dtypes
import concourse.mybir as mybir
import concourse.tile as tile
from concourse import bacc
from concourse.masks import make_identity
from contextlib import ExitStack

F32 = mybir.dt.float32
BF16 = mybir.dt.bfloat16
FP8 = mybir.dt.float8e4
I16 = mybir.dt.int16
FP16 = mybir.dt.float16
EXP = mybir.ActivationFunctionType.Exp
DR = mybir.MatmulPerfMode.DoubleRow
MUL = mybir.AluOpType.mult
ADD = mybir.AluOpType.add
SUB = mybir.AluOpType.subtract

P = 128
HID = 256
NHEAD = 4
DHEAD = 64
NQ = 1024  # queries per core
NK = 2048  # keys (full)
QBLK = 256
NQB = NQ // QBLK  # 4
NKT = NK // P  # 16
NCT = HID // P  # 2

# ktile assignment to exp engines: Act | DVE | Pool
NA = 10
ND = 6
NP = NKT - NA - ND  # 0 (gpsimd cannot access PSUM)

LOGM = -30.0   # masked-out additive bias (exact in e4m3); exp(-30+s) ~ 0
SHIFT = -4.0   # uniform logit shift (softmax-invariant): max score ~8.2 stays accurate
SCH_A = 184.664965  # 2^7 / ln 2
SCH_B = 16253.0     # 127*2^7 - 3 (Schraudolph bias; constant factor cancels)

# scr free-layout (fp8 elements per partition):
#   q region: (t, par) x (hi, lo) -> 8 x 1024, then logM double buffer 2 x 4096
SCR_Q = 8 * NQ            # 8192
SCR_W = SCR_Q + 2 * NKT * QBLK  # 16384
# kt8 free-layout per t-half: [ K_hi 2048 | K_lo 2048 | identity 128 ]
KT8_W = 2 * NK + P  # 4224


def _q_off(t: int, par: int, lo: int, qb: int) -> int:
    return ((2 * t + par) * 2 + lo) * NQ + qb * QBLK


def _m_off(qb: int, kt: int) -> int:
    return SCR_Q + (qb % 2) * NKT * QBLK + kt * QBLK


def build(with_attn_mask: bool) -> bass.Bass:
    del with_attn_mask  # attn mask folded into logm8 on host
    nc = bacc.Bacc()
    xqT = nc.declare_dram_parameter("xqT", [HID, NQ], BF16, isOutput=False)
    xkT = nc.declare_dram_parameter("xkT", [HID, NK], BF16, isOutput=False)
    wqT = nc.declare_dram_parameter("wqT", [HID, HID], BF16, isOutput=False)
    wkT = nc.declare_dram_parameter("wkT", [HID, HID], BF16, isOutput=False)
    wvT = nc.declare_dram_parameter("wvT", [HID, HID], BF16, isOutput=False)
    woT = nc.declare_dram_parameter("woT", [HID, HID], BF16, isOutput=False)
    logm8 = nc.declare_dram_parameter("logm8", [NQB, P, NKT, QBLK], FP8, isOutput=False)
    out_d = nc.declare_dram_parameter("out", [NQ, HID], F32, isOutput=True)

    with tile.TileContext(nc) as tc, ExitStack() as ctx:
        const = ctx.enter_context(tc.tile_pool(name="const", bufs=1))
        big = ctx.enter_context(tc.tile_pool(name="big", bufs=1))
        p16p = ctx.enter_context(tc.tile_pool(name="p16p", bufs=2))
        anp = ctx.enter_context(tc.tile_pool(name="anp", bufs=2))
        wrk = ctx.enter_context(tc.tile_pool(name="wrk", bufs=4))
        outp = ctx.enter_context(tc.tile_pool(name="outp", bufs=3))

        # --- weights + activations ---
        def load2(name, src, width):
            ts = []
            for t in range(2):
                tl = const.tile([P, width], BF16, tag=f"{name}{t}", name=f"{name}{t}")
                nc.sync.dma_start(out=tl, in_=src[t * P : (t + 1) * P, :])
                ts.append(tl)
            return ts

        wq_sb = load2("wq", wqT, HID)
        xq_sb = [big.tile([P, NQ], BF16, tag=f"xq{t}", name=f"xq{t}") for t in range(2)]
        xk_sb = [big.tile([P, NK], BF16, tag=f"xk{t}", name=f"xk{t}") for t in range(2)]
        for nb in range(NQ // 512):
            for t in range(2):
                nc.sync.dma_start(
                    out=xq_sb[t][:, nb * 512 : (nb + 1) * 512],
                    in_=xqT[t * P : (t + 1) * P, nb * 512 : (nb + 1) * 512],
                )
        wk_sb = load2("wk", wkT, HID)
        for nb in range(NK // 512):
            for t in range(2):
                nc.sync.dma_start(
                    out=xk_sb[t][:, nb * 512 : (nb + 1) * 512],
                    in_=xkT[t * P : (t + 1) * P, nb * 512 : (nb + 1) * 512],
                )
        wv_sb = load2("wv", wvT, HID)
        wo_sb = load2("wo", woT, HID)

        ident16 = const.tile([P, P], BF16, tag="id16", name="id16")
        make_identity(nc, ident16)

        # fused scores scratch: q hi/lo halves (zero-padded) + logM double buffer
        scr = big.tile([P, SCR_W], FP8, tag="scr", name="scr")
        # kt8 per t-half: [K_hi | K_lo | identity] fp8
        kt8 = [big.tile([P, KT8_W], FP8, tag=f"kt8_{t}", name=f"kt8_{t}") for t in range(2)]
        for t in range(2):
            idap = kt8[t][:, 2 * NK : 2 * NK + P]
            nc.gpsimd.memset(idap, 0.0)
            nc.gpsimd.affine_select(
                out=idap, in_=idap, compare_op=mybir.AluOpType.not_equal,
                fill=32.0, base=0, pattern=[[-1, P]], channel_multiplier=1,
            )

        # zero the dead half-rows of q regions (even heads live in rows 0:64,
        # odd heads in rows 64:128; the other half must be 0 for the fused DR)
        for t in range(2):
            for lo in range(2):
                o0 = _q_off(t, 0, lo, 0)
                o1 = _q_off(t, 1, lo, 0)
                nc.scalar.memzero(scr[DHEAD:P, o0 : o0 + NQ])
                nc.scalar.memzero(scr[0:DHEAD, o1 : o1 + NQ])

        # V (bf16) with denominator ones-column
        v16 = big.tile([P, NKT, NHEAD, DHEAD + 1], BF16, tag="v16", name="v16")
        nc.gpsimd.memset(v16[:, :, :, DHEAD : DHEAD + 1], 1.0)

        with ExitStack() as pctx:
            ps_pr = pctx.enter_context(tc.tile_pool(name="ps_pr", bufs=8, space="PSUM"))
            # Q projection -> q_hi fp8 + q_lo fp8 (head-parity split, zero halves)
            for t in range(2):
                for nb in range(NQ // 512):
                    ps = ps_pr.tile([P, 512], F32, tag="pr", name="pr")
                    for ct in range(NCT):
                        nc.tensor.matmul(
                            ps,
                            lhsT=wq_sb[ct][:, t * P : (t + 1) * P],
                            rhs=xq_sb[ct][:, nb * 512 : (nb + 1) * 512],
                            start=(ct == 0),
                            stop=(ct == NCT - 1),
                        )
                    for par, rows in ((0, slice(0, DHEAD)), (1, slice(DHEAD, P))):
                        hi = _q_off(t, par, 0, 0) + nb * 512
                        lo = _q_off(t, par, 1, 0) + nb * 512
                        nc.vector.tensor_copy(scr[rows, hi : hi + 512], ps[rows, :])
                        nc.vector.scalar_tensor_tensor(
                            scr[rows, lo : lo + 512],
                            ps[rows, :],
                            1.0,
                            scr[rows, hi : hi + 512],
                            op0=MUL,
                            op1=SUB,
                        )
            # K projection -> k_hi + k_lo fp8
            for t in range(2):
                for nb in range(NK // 512):
                    ps = ps_pr.tile([P, 512], F32, tag="pr", name="pr")
                    for ct in range(NCT):
                        nc.tensor.matmul(
                            ps,
                            lhsT=wk_sb[ct][:, t * P : (t + 1) * P],
                            rhs=xk_sb[ct][:, nb * 512 : (nb + 1) * 512],
                            start=(ct == 0),
                            stop=(ct == NCT - 1),
                        )
                    c0 = nb * 512
                    nc.scalar.copy(kt8[t][:, c0 : c0 + 512], ps)
                    nc.vector.scalar_tensor_tensor(
                        kt8[t][:, NK + c0 : NK + c0 + 512],
                        ps,
                        1.0,
                        kt8[t][:, c0 : c0 + 512],
                        op0=MUL,
                        op1=SUB,
                    )
            # V projection -> v16 bf16
            for kt in range(NKT):
                ps = ps_pr.tile([P, HID], F32, tag="pr", name="pr")
                for ct in range(NCT):
                    nc.tensor.matmul(
                        ps,
                        lhsT=xk_sb[ct][:, kt * P : (kt + 1) * P],
                        rhs=wv_sb[ct],
                        start=(ct == 0),
                        stop=(ct == NCT - 1),
                    )
                psr = ps.rearrange("p (h d) -> p h d", h=NHEAD)
                if kt % 2 == 0:
                    nc.vector.tensor_copy(v16[:, kt, :, 0:DHEAD], psr)
                else:
                    nc.scalar.copy(v16[:, kt, :, 0:DHEAD], psr)

        # --- attention ---
        with ExitStack() as actx:
            ps_sc = actx.enter_context(tc.tile_pool(name="ps_sc", bufs=3, space="PSUM"))
            ps_ao = actx.enter_context(tc.tile_pool(name="ps_ao", bufs=2, space="PSUM"))

            GROUPS = [list(range(g * 4, (g + 1) * 4)) for g in range(NKT // 4)]

            def seg_engine(kt):
                return 0 if kt < NA else (1 if kt < NA + ND else 2)

            def emit_score_group(qb, h, p16, grp, seg_fn=None):
                """PE: 2 fused DR matmuls per ktile; Act/DVE/Pool drain to P."""
                seg = seg_fn or seg_engine
                t, par = h // 2, h % 2
                qhi = _q_off(t, par, 0, qb)
                qlo = _q_off(t, par, 1, qb)
                ps = ps_sc.tile([P, 4, QBLK], F32, tag="sc", name="sc")
                for j, kt in enumerate(grp):
                    # DR1: K_hi' q_hi + K_hi' q_lo
                    lhs1 = bass.AP(
                        kt8[t].tensor,
                        kt8[t].offset + kt * P,
                        [[KT8_W, P], [0, 2], [1, P]],
                    )
                    rhs1 = bass.AP(
                        scr.tensor,
                        scr.offset + qhi,
                        [[SCR_W, P], [qlo - qhi, 2], [1, QBLK]],
                    )
                    nc.tensor.matmul(
                        ps[:, j, :], lhsT=lhs1, rhs=rhs1,
                        start=True, stop=False, perf_mode=DR,
                    )
                    # DR2: K_lo' q_hi + I * logM
                    lhs2 = bass.AP(
                        kt8[t].tensor,
                        kt8[t].offset + NK + kt * P,
                        [[KT8_W, P], [NK - kt * P, 2], [1, P]],
                    )
                    rhs2 = bass.AP(
                        scr.tensor,
                        scr.offset + qhi,
                        [[SCR_W, P], [_m_off(qb, kt) - qhi, 2], [1, QBLK]],
                    )
                    nc.tensor.matmul(
                        ps[:, j, :], lhsT=lhs2, rhs=rhs2,
                        start=False, stop=True, perf_mode=DR,
                    )
                # drain by engine segment (kt ranges are contiguous)
                j0 = 0
                while j0 < 4:
                    eng = seg(grp[j0])
                    j1 = j0
                    while j1 < 4 and seg(grp[j1]) == eng:
                        j1 += 1
                    src = ps[:, j0:j1, :]
                    dst = p16[:, grp[j0] : grp[j1 - 1] + 1, :]
                    if eng == 0:
                        nc.scalar.activation(dst, src, EXP, scale=float(1.0 / SCH_A))
                    else:
                        nc.vector.tensor_scalar(
                            dst.bitcast(I16), src, SCH_B, None, ADD
                        )
                    j0 = j1

            def emit_a_qt(st, qt):
                """PE: A(qt) = P^T V (+denom col); DVE recip; Pool normalize."""
                qb, h, p16, anorm, ap_t = st
                ap_ = ap_t[:, qt, :]
                qsl = slice(qt * P, (qt + 1) * P)
                for kt in range(NKT):
                    nc.tensor.matmul(
                        ap_,
                        lhsT=p16[:, kt, qsl],
                        rhs=v16[:, kt, h, :],
                        start=(kt == 0),
                        stop=(kt == NKT - 1),
                    )
                rec = wrk.tile([P, 1], F32, tag="rec", name="rec")
                nc.vector.reciprocal(rec, ap_[:, DHEAD : DHEAD + 1])
                nc.vector.tensor_scalar(
                    anorm[:, qt, h * DHEAD : (h + 1) * DHEAD],
                    ap_[:, 0:DHEAD],
                    rec,
                    None,
                    MUL,
                )

            def emit_oproj_qt(qb, anorm, qt):
                # anorm^T via DMA transpose (no PE/DVE cost), then bf16 matmul;
                # o accum shares the ps_ao union bank with the A tile
                bank = ps_ao.tile([P, 512], F32, tag="ao", name="ao")
                o_ps = bank[:, 0:256]
                atts = []
                for ct in range(NCT):
                    att = wrk.tile([P, P], BF16, tag=f"att{ct}", name=f"att{ct}")
                    nc.sync.dma_start_transpose(
                        att, anorm[:, qt, ct * P : (ct + 1) * P]
                    )
                    atts.append(att)
                for ct in range(NCT):
                    nc.tensor.matmul(
                        o_ps, lhsT=atts[ct], rhs=wo_sb[ct],
                        start=(ct == 0), stop=(ct == NCT - 1),
                    )
                ob = outp.tile([P, HID], F32, tag="ob", name="ob")
                nc.vector.tensor_copy(ob, o_ps)
                q0 = qb * QBLK + qt * P
                nc.sync.dma_start(out=out_d[q0 : q0 + P, :], in_=ob)

            # Software pipeline: A(i) (two qt halves) and O-proj quarters are
            # interleaved between score groups of step i+1 so PE never waits
            # on exp drains and the clock ramp is preserved.
            steps = [(qb, h) for qb in range(NQB) for h in range(NHEAD)]
            pend_a = None  # (qb, h, p16, anorm, ap_t)
            pend_o = []    # [(qb, anorm, qt), ...]
            anorms = {}
            def fetch_logm(qb):
                mb = _m_off(qb, 0)
                nc.sync.dma_start(
                    out=scr[:, mb : mb + NKT * QBLK],
                    in_=logm8[qb].rearrange("p t q -> p (t q)"),
                )

            fetch_logm(0)
            for qb, h in steps:
                if h == 0:
                    anorms[qb] = anp.tile([P, 2, HID], BF16, tag="an", name="an")
                if h == 1 and qb + 1 < NQB:
                    fetch_logm(qb + 1)
                p16 = p16p.tile([P, NKT, QBLK], BF16, tag="p16", name="p16")
                last = qb == NQB - 1 and h == NHEAD - 1
                seg_fn = (lambda kt: (0, 1, 0, 1, 0, 1, 0, 1, 0, 1, 0, 1, 0, 1, 0, 1)[kt]) if last else None
                emit_score_group(qb, h, p16, GROUPS[0], seg_fn)
                emit_score_group(qb, h, p16, GROUPS[1], seg_fn)
                if pend_a is not None:
                    emit_a_qt(pend_a, 0)
                emit_score_group(qb, h, p16, GROUPS[2], seg_fn)
                if pend_a is not None:
                    emit_a_qt(pend_a, 1)
                emit_score_group(qb, h, p16, GROUPS[3], seg_fn)
                if pend_o:
                    emit_oproj_qt(*pend_o.pop(0))
                abank = ps_ao.tile([P, 512], F32, tag="ao", name="ao")
                ap_t = abank[:, 256 : 256 + 2 * (DHEAD + 1)].rearrange(
                    "p (a b) -> p a b", a=2
                )
                pend_a = (qb, h, p16, anorms[qb], ap_t)
                if h == 1 and qb > 0:
                    an = anorms.pop(qb - 1)
                    pend_o += [(qb - 1, an, 0), (qb - 1, an, 1)]
                if last:
                    # drain tail immediately, A interleaved per group already
                    emit_a_qt(pend_a, 0)
                    emit_a_qt(pend_a, 1)
                    pend_a = None
                    an = anorms.pop(NQB - 1)
                    emit_oproj_qt(NQB - 1, an, 0)
                    emit_oproj_qt(NQB - 1, an, 1)
    nc.compile()
    return nc


_NC_CACHE = {}
_last_in_maps = None


def _get_nc(with_attn_mask: bool = False) -> bass.Bass:
    if with_attn_mask not in _NC_CACHE:
        _NC_CACHE[with_attn_mask] = build(with_attn_mask)
    return _NC_CACHE[with_attn_mask]


def kernel(q_hidden_states, k_hidden_states, attention_mask, align_mask, Wq, Wk, Wv, Wo):
    from concourse.bass_utils import run_bass_kernel_spmd

    q_hidden_states = np.asarray(q_hidden_states, np.float32)
    k_hidden_states = np.asarray(k_hidden_states, np.float32)
    attention_mask = np.asarray(attention_mask, np.float32)
    align_mask = np.asarray(align_mask)
    B, Q, _ = q_hidden_states.shape
    qh_len = Q // 2  # 1024

    use_mask = bool(np.any(attention_mask))
    nc = _get_nc(False)

    bf16 = ml_dtypes.bfloat16
    f8 = ml_dtypes.float8_e4m3
    s8 = np.float32(np.sqrt(SCH_A) / np.sqrt(8.0))
    wq = np.ascontiguousarray(np.asarray(Wq, np.float32).T * s8).astype(bf16)
    wk = np.ascontiguousarray(np.asarray(Wk, np.float32).T * s8).astype(bf16)
    wv = np.ascontiguousarray(np.asarray(Wv, np.float32).T).astype(bf16)
    wo = np.ascontiguousarray(np.asarray(Wo, np.float32).T).astype(bf16)

    in_maps = []
    for core in range(8):
        b, qh = divmod(core, 2)
        qsl = slice(qh * qh_len, (qh + 1) * qh_len)
        logm = np.where(align_mask[b, :, qsl] == 0, np.float32(LOGM), np.float32(SHIFT))
        if use_mask:
            amk = attention_mask[b, 0, qsl, :].T  # [K, Q-slice]
            logm = np.maximum(logm + amk, np.float32(LOGM))
        logm = logm * np.float32(SCH_A / 32.0)
        # [K, Qh] -> [qb, p, t, q] with k = t*128 + p, qcol = qb*256 + q
        lm = logm.reshape(NKT, P, NQB, QBLK).transpose(2, 1, 0, 3)
        m = {
            "xqT": np.ascontiguousarray(q_hidden_states[b, qsl].T).astype(bf16),
            "xkT": np.ascontiguousarray(k_hidden_states[b].T).astype(bf16),
            "wqT": wq,
            "wkT": wk,
            "wvT": wv,
            "woT": wo,
            "logm8": np.ascontiguousarray(lm).astype(f8),
        }
        in_maps.append(m)

    global _last_in_maps
    _last_in_maps = in_maps
    res = run_bass_kernel_spmd(nc, in_maps, list(range(8))).results
    out = np.empty((B, Q, HID), np.float32)
    for core in range(8):
        b, qh = divmod(core, 2)
        out[b, qh * qh_len : (qh + 1) * qh_len] = res[core]["out"]
    return out


# revision 7
# speedup vs baseline: 1.0650x; 1.0154x over previous
"""KgAdapterCrossAttention kernel for 8 trn2 NeuronCores — v2.

Sharding: core = (batch b, query-half qh). Each core: 1024 queries x 2048 keys,
4 heads of 64 dims.

Techniques vs baseline (fp32 everywhere):
  - projections and A/O matmuls in bf16 (PE 1 cycle/row vs fp32's 4),
  - scores via error-compensated fp8e4m3 DoubleRow: with q = q_hi + q_lo and
    k = k_hi + k_lo (residual quantization), two DR matmuls per (head, ktile)
    compute K_hi'(q_hi + q_lo) and K_lo'q_hi + I*logM at 0.5 cycles/row —
    near-full-precision scores at half the bf16 PE cost, with the align-mask
    add fused into the last DR slot for free,
  - exp split across three engines: Act (native Exp -> bf16 P) plus DVE and
    Pool (Schraudolph fast-exp: int16 bits = S*184.665 + B, bitcast to bf16;
    the fused logM (-30) makes masked entries ~2^-40),
  - per-head softmax denominator from a ones-column appended to V,
  - O projection in bf16 after PE transposes.

Softmax has no max-subtraction; all logits carry a uniform -4 shift (softmax
invariant) so exp stays small; Schraudolph's constant factor cancels in the
p/sum(p) ratio.
"""

import os
import sys

import numpy as np

try:
    import concourse.bass as bass
except ImportError:
    for _p in ("/opt/trn_rl_repo", os.path.expanduser("~/.axon_site/_ro/trn_rl_repo")):
        if os.path.isdir(_p) and _p not in sys.path:
            sys.path.insert(0, _p)
    import concourse.bass as bass

import ml_dtypes
import concourse.mybir as mybir
import concourse.tile as tile
from concourse import bacc
from concourse.masks import make_identity
from contextlib import ExitStack

F32 = mybir.dt.float32
BF16 = mybir.dt.bfloat16
FP8 = mybir.dt.float8e4
I16 = mybir.dt.int16
FP16 = mybir.dt.float16
EXP = mybir.ActivationFunctionType.Exp
DR = mybir.MatmulPerfMode.DoubleRow
MUL = mybir.AluOpType.mult
ADD = mybir.AluOpType.add
SUB = mybir.AluOpType.subtract

P = 128
HID = 256
NHEAD = 4
DHEAD = 64
NQ = 1024  # queries per core
NK = 2048  # keys (full)
QBLK = 256
NQB = NQ // QBLK  # 4
NKT = NK // P  # 16
NCT = HID // P  # 2

# ktile assignment to exp engines: Act | DVE | Pool
NA = 11
ND = 5
NP = NKT - NA - ND  # 0 (gpsimd cannot access PSUM)

LOGM = -30.0   # masked-out additive bias (exact in e4m3); exp(-30+s) ~ 0
SHIFT = -4.0   # uniform logit shift (softmax-invariant): max score ~8.2 stays accurate
SCH_A = 184.664965  # 2^7 / ln 2
SCH_B = 16253.0     # 127*2^7 - 3 (Schraudolph bias; constant factor cancels)

# scr free-layout (fp8 elements per partition):
#   q region: (t, par) x (hi, lo) -> 8 x 1024, then logM double buffer 2 x 4096
SCR_Q = 8 * NQ            # 8192
SCR_W = SCR_Q + 2 * NKT * QBLK  # 16384
# kt8 free-layout per t-half: [ K_hi 2048 | K_lo 2048 | identity 128 ]
KT8_W = 2 * NK + P  # 4224


def _q_off(t: int, par: int, lo: int, qb: int) -> int:
    return ((2 * t + par) * 2 + lo) * NQ + qb * QBLK


def _m_off(qb: int, kt: int) -> int:
    return SCR_Q + (qb % 2) * NKT * QBLK + kt * QBLK


def build(with_attn_mask: bool) -> bass.Bass:
    del with_attn_mask  # attn mask folded into logm8 on host
    nc = bacc.Bacc()
    xqT = nc.declare_dram_parameter("xqT", [HID, NQ], BF16, isOutput=False)
    xkT = nc.declare_dram_parameter("xkT", [HID, NK], BF16, isOutput=False)
    wqT = nc.declare_dram_parameter("wqT", [HID, HID], BF16, isOutput=False)
    wkT = nc.declare_dram_parameter("wkT", [HID, HID], BF16, isOutput=False)
    wvT = nc.declare_dram_parameter("wvT", [HID, HID], BF16, isOutput=False)
    woT = nc.declare_dram_parameter("woT", [HID, HID], BF16, isOutput=False)
    logm8 = nc.declare_dram_parameter("logm8", [NQB, P, NKT, QBLK], FP8, isOutput=False)
    out_d = nc.declare_dram_parameter("out", [NQ, HID], F32, isOutput=True)

    with tile.TileContext(nc) as tc, ExitStack() as ctx:
        const = ctx.enter_context(tc.tile_pool(name="const", bufs=1))
        big = ctx.enter_context(tc.tile_pool(name="big", bufs=1))
        p16p = ctx.enter_context(tc.tile_pool(name="p16p", bufs=2))
        anp = ctx.enter_context(tc.tile_pool(name="anp", bufs=2))
        wrk = ctx.enter_context(tc.tile_pool(name="wrk", bufs=4))
        outp = ctx.enter_context(tc.tile_pool(name="outp", bufs=3))

        # --- weights + activations ---
        def load2(name, src, width):
            ts = []
            for t in range(2):
                tl = const.tile([P, width], BF16, tag=f"{name}{t}", name=f"{name}{t}")
                nc.sync.dma_start(out=tl, in_=src[t * P : (t + 1) * P, :])
                ts.append(tl)
            return ts

        wq_sb = load2("wq", wqT, HID)
        xq_sb = [big.tile([P, NQ], BF16, tag=f"xq{t}", name=f"xq{t}") for t in range(2)]
        xk_sb = [big.tile([P, NK], BF16, tag=f"xk{t}", name=f"xk{t}") for t in range(2)]
        for nb in range(NQ // 512):
            for t in range(2):
                nc.sync.dma_start(
                    out=xq_sb[t][:, nb * 512 : (nb + 1) * 512],
                    in_=xqT[t * P : (t + 1) * P, nb * 512 : (nb + 1) * 512],
                )
        wk_sb = load2("wk", wkT, HID)
        for nb in range(NK // 512):
            for t in range(2):
                nc.sync.dma_start(
                    out=xk_sb[t][:, nb * 512 : (nb + 1) * 512],
                    in_=xkT[t * P : (t + 1) * P, nb * 512 : (nb + 1) * 512],
                )
        wv_sb = load2("wv", wvT, HID)
        wo_sb = load2("wo", woT, HID)

        # fused scores scratch: q hi/lo halves (zero-padded) + logM double buffer
        scr = big.tile([P, SCR_W], FP8, tag="scr", name="scr")
        # kt8 per t-half: [K_hi | K_lo | identity] fp8
        kt8 = [big.tile([P, KT8_W], FP8, tag=f"kt8_{t}", name=f"kt8_{t}") for t in range(2)]
        for t in range(2):
            idap = kt8[t][:, 2 * NK : 2 * NK + P]
            nc.gpsimd.memset(idap, 0.0)
            nc.gpsimd.affine_select(
                out=idap, in_=idap, compare_op=mybir.AluOpType.not_equal,
                fill=32.0, base=0, pattern=[[-1, P]], channel_multiplier=1,
            )

        # zero the dead half-rows of q regions (even heads live in rows 0:64,
        # odd heads in rows 64:128; the other half must be 0 for the fused DR)
        for t in range(2):
            for lo in range(2):
                o0 = _q_off(t, 0, lo, 0)
                o1 = _q_off(t, 1, lo, 0)
                nc.gpsimd.memset(scr[DHEAD:P, o0 : o0 + NQ], 0.0)
                nc.gpsimd.memset(scr[0:DHEAD, o1 : o1 + NQ], 0.0)

        # V (bf16) with denominator ones-column
        v16 = big.tile([P, NKT, NHEAD, DHEAD + 1], BF16, tag="v16", name="v16")
        nc.gpsimd.memset(v16[:, :, :, DHEAD : DHEAD + 1], 1.0)

        with ExitStack() as pctx:
            ps_pr = pctx.enter_context(tc.tile_pool(name="ps_pr", bufs=8, space="PSUM"))
            # Q projection -> q_hi fp8 + q_lo fp8 (head-parity split, zero halves)
            for t in range(2):
                for nb in range(NQ // 512):
                    ps = ps_pr.tile([P, 512], F32, tag="pr", name="pr")
                    for ct in range(NCT):
                        nc.tensor.matmul(
                            ps,
                            lhsT=wq_sb[ct][:, t * P : (t + 1) * P],
                            rhs=xq_sb[ct][:, nb * 512 : (nb + 1) * 512],
                            start=(ct == 0),
                            stop=(ct == NCT - 1),
                        )
                    for par, rows in ((0, slice(0, DHEAD)), (1, slice(DHEAD, P))):
                        hi = _q_off(t, par, 0, 0) + nb * 512
                        lo = _q_off(t, par, 1, 0) + nb * 512
                        nc.scalar.copy(scr[rows, hi : hi + 512], ps[rows, :])
                        nc.vector.scalar_tensor_tensor(
                            scr[rows, lo : lo + 512],
                            ps[rows, :],
                            1.0,
                            scr[rows, hi : hi + 512],
                            op0=MUL,
                            op1=SUB,
                        )
            # K projection -> k_hi + k_lo fp8
            for t in range(2):
                for nb in range(NK // 512):
                    ps = ps_pr.tile([P, 512], F32, tag="pr", name="pr")
                    for ct in range(NCT):
                        nc.tensor.matmul(
                            ps,
                            lhsT=wk_sb[ct][:, t * P : (t + 1) * P],
                            rhs=xk_sb[ct][:, nb * 512 : (nb + 1) * 512],
                            start=(ct == 0),
                            stop=(ct == NCT - 1),
                        )
                    c0 = nb * 512
                    nc.scalar.copy(kt8[t][:, c0 : c0 + 512], ps)
                    nc.vector.scalar_tensor_tensor(
                        kt8[t][:, NK + c0 : NK + c0 + 512],
                        ps,
                        1.0,
                        kt8[t][:, c0 : c0 + 512],
                        op0=MUL,
                        op1=SUB,
                    )
            # V projection -> v16 bf16
            for kt in range(NKT):
                ps = ps_pr.tile([P, HID], F32, tag="pr", name="pr")
                for ct in range(NCT):
                    nc.tensor.matmul(
                        ps,
                        lhsT=xk_sb[ct][:, kt * P : (kt + 1) * P],
                        rhs=wv_sb[ct],
                        start=(ct == 0),
                        stop=(ct == NCT - 1),
                    )
                psr = ps.rearrange("p (h d) -> p h d", h=NHEAD)
                if kt % 2 == 0:
                    nc.vector.tensor_copy(v16[:, kt, :, 0:DHEAD], psr)
                else:
                    nc.scalar.copy(v16[:, kt, :, 0:DHEAD], psr)

        # --- attention ---
        with ExitStack() as actx:
            ps_sc = actx.enter_context(tc.tile_pool(name="ps_sc", bufs=3, space="PSUM"))
            ps_ao = actx.enter_context(tc.tile_pool(name="ps_ao", bufs=2, space="PSUM"))

            GROUPS = [list(range(g * 4, (g + 1) * 4)) for g in range(NKT // 4)]

            def seg_engine(kt):
                return 0 if kt < NA else (1 if kt < NA + ND else 2)

            def emit_score_group(qb, h, p16, grp, seg_fn=None):
                """PE: 2 fused DR matmuls per ktile; Act/DVE/Pool drain to P."""
                seg = seg_fn or seg_engine
                t, par = h // 2, h % 2
                qhi = _q_off(t, par, 0, qb)
                qlo = _q_off(t, par, 1, qb)
                ps = ps_sc.tile([P, 4, QBLK], F32, tag="sc", name="sc")
                for j, kt in enumerate(grp):
                    # DR1: K_hi' q_hi + K_hi' q_lo
                    lhs1 = bass.AP(
                        kt8[t].tensor,
                        kt8[t].offset + kt * P,
                        [[KT8_W, P], [0, 2], [1, P]],
                    )
                    rhs1 = bass.AP(
                        scr.tensor,
                        scr.offset + qhi,
                        [[SCR_W, P], [qlo - qhi, 2], [1, QBLK]],
                    )
                    nc.tensor.matmul(
                        ps[:, j, :], lhsT=lhs1, rhs=rhs1,
                        start=True, stop=False, perf_mode=DR,
                    )
                    # DR2: K_lo' q_hi + I * logM
                    lhs2 = bass.AP(
                        kt8[t].tensor,
                        kt8[t].offset + NK + kt * P,
                        [[KT8_W, P], [NK - kt * P, 2], [1, P]],
                    )
                    rhs2 = bass.AP(
                        scr.tensor,
                        scr.offset + qhi,
                        [[SCR_W, P], [_m_off(qb, kt) - qhi, 2], [1, QBLK]],
                    )
                    nc.tensor.matmul(
                        ps[:, j, :], lhsT=lhs2, rhs=rhs2,
                        start=False, stop=True, perf_mode=DR,
                    )
                # drain by engine segment (kt ranges are contiguous)
                j0 = 0
                while j0 < 4:
                    eng = seg(grp[j0])
                    j1 = j0
                    while j1 < 4 and seg(grp[j1]) == eng:
                        j1 += 1
                    src = ps[:, j0:j1, :]
                    dst = p16[:, grp[j0] : grp[j1 - 1] + 1, :]
                    if eng == 0:
                        nc.scalar.activation(dst, src, EXP, scale=float(1.0 / SCH_A))
                    else:
                        nc.vector.tensor_scalar(
                            dst.bitcast(I16), src, SCH_B, None, ADD
                        )
                    j0 = j1

            def emit_a_qt(st, qt):
                """PE: A(qt) = P^T V (+denom col); DVE recip; Pool normalize."""
                qb, h, p16, anorm, ap_t = st
                ap_ = ap_t[:, qt, :]
                qsl = slice(qt * P, (qt + 1) * P)
                for kt in range(NKT):
                    nc.tensor.matmul(
                        ap_,
                        lhsT=p16[:, kt, qsl],
                        rhs=v16[:, kt, h, :],
                        start=(kt == 0),
                        stop=(kt == NKT - 1),
                    )
                rec = wrk.tile([P, 1], F32, tag="rec", name="rec")
                nc.vector.reciprocal(rec, ap_[:, DHEAD : DHEAD + 1])
                nc.vector.tensor_scalar(
                    anorm[:, qt, h * DHEAD : (h + 1) * DHEAD],
                    ap_[:, 0:DHEAD],
                    rec,
                    None,
                    MUL,
                )

            def emit_oproj_qt(qb, anorm, qt):
                # anorm^T via DMA transpose (no PE/DVE cost), then bf16 matmul;
                # o accum shares the ps_ao union bank with the A tile
                bank = ps_ao.tile([P, 512], F32, tag="ao", name="ao")
                o_ps = bank[:, 0:256]
                atts = []
                for ct in range(NCT):
                    att = wrk.tile([P, P], BF16, tag=f"att{ct}", name=f"att{ct}")
                    nc.sync.dma_start_transpose(
                        att, anorm[:, qt, ct * P : (ct + 1) * P]
                    )
                    atts.append(att)
                for ct in range(NCT):
                    nc.tensor.matmul(
                        o_ps, lhsT=atts[ct], rhs=wo_sb[ct],
                        start=(ct == 0), stop=(ct == NCT - 1),
                    )
                ob = outp.tile([P, HID], F32, tag="ob", name="ob")
                nc.vector.tensor_copy(ob, o_ps)
                q0 = qb * QBLK + qt * P
                nc.sync.dma_start(out=out_d[q0 : q0 + P, :], in_=ob)

            # Software pipeline: A(i) (two qt halves) and O-proj quarters are
            # interleaved between score groups of step i+1 so PE never waits
            # on exp drains and the clock ramp is preserved.
            steps = [(qb, h) for qb in range(NQB) for h in range(NHEAD)]
            pend_a = None  # (qb, h, p16, anorm, ap_t)
            pend_o = []    # [(qb, anorm, qt), ...]
            anorms = {}
            def fetch_logm(qb):
                mb = _m_off(qb, 0)
                nc.sync.dma_start(
                    out=scr[:, mb : mb + NKT * QBLK],
                    in_=logm8[qb].rearrange("p t q -> p (t q)"),
                )

            fetch_logm(0)
            for qb, h in steps:
                if h == 0:
                    anorms[qb] = anp.tile([P, 2, HID], BF16, tag="an", name="an")
                if h == 1 and qb + 1 < NQB:
                    fetch_logm(qb + 1)
                p16 = p16p.tile([P, NKT, QBLK], BF16, tag="p16", name="p16")
                last = qb == NQB - 1 and h == NHEAD - 1
                seg_fn = (lambda kt: (0, 1, 0, 1, 0, 1, 0, 1, 0, 1, 0, 1, 0, 1, 0, 1)[kt]) if last else None
                emit_score_group(qb, h, p16, GROUPS[0], seg_fn)
                emit_score_group(qb, h, p16, GROUPS[1], seg_fn)
                if pend_a is not None:
                    emit_a_qt(pend_a, 0)
                emit_score_group(qb, h, p16, GROUPS[2], seg_fn)
                if pend_a is not None:
                    emit_a_qt(pend_a, 1)
                emit_score_group(qb, h, p16, GROUPS[3], seg_fn)
                if pend_o:
                    emit_oproj_qt(*pend_o.pop(0))
                abank = ps_ao.tile([P, 512], F32, tag="ao", name="ao")
                ap_t = abank[:, 256 : 256 + 2 * (DHEAD + 1)].rearrange(
                    "p (a b) -> p a b", a=2
                )
                pend_a = (qb, h, p16, anorms[qb], ap_t)
                if h == 1 and qb > 0:
                    an = anorms.pop(qb - 1)
                    pend_o += [(qb - 1, an, 0), (qb - 1, an, 1)]
                if last:
                    # drain tail immediately, A interleaved per group already
                    emit_a_qt(pend_a, 0)
                    emit_a_qt(pend_a, 1)
                    pend_a = None
                    an = anorms.pop(NQB - 1)
                    emit_oproj_qt(NQB - 1, an, 0)
                    emit_oproj_qt(NQB - 1, an, 1)
    nc.compile()
    return nc


_NC_CACHE = {}
_last_in_maps = None


def _get_nc(with_attn_mask: bool = False) -> bass.Bass:
    if with_attn_mask not in _NC_CACHE:
        _NC_CACHE[with_attn_mask] = build(with_attn_mask)
    return _NC_CACHE[with_attn_mask]


def kernel(q_hidden_states, k_hidden_states, attention_mask, align_mask, Wq, Wk, Wv, Wo):
    from concourse.bass_utils import run_bass_kernel_spmd

    q_hidden_states = np.asarray(q_hidden_states, np.float32)
    k_hidden_states = np.asarray(k_hidden_states, np.float32)
    attention_mask = np.asarray(attention_mask, np.float32)
    align_mask = np.asarray(align_mask)
    B, Q, _ = q_hidden_states.shape
    qh_len = Q // 2  # 1024

    use_mask = bool(np.any(attention_mask))
    nc = _get_nc(False)

    bf16 = ml_dtypes.bfloat16
    f8 = ml_dtypes.float8_e4m3
    s8 = np.float32(np.sqrt(SCH_A) / np.sqrt(8.0))
    wq = np.ascontiguousarray(np.asarray(Wq, np.float32).T * s8).astype(bf16)
    wk = np.ascontiguousarray(np.asarray(Wk, np.float32).T * s8).astype(bf16)
    wv = np.ascontiguousarray(np.asarray(Wv, np.float32).T).astype(bf16)
    wo = np.ascontiguousarray(np.asarray(Wo, np.float32).T).astype(bf16)

    in_maps = []
    for core in range(8):
        b, qh = divmod(core, 2)
        qsl = slice(qh * qh_len, (qh + 1) * qh_len)
        logm = np.where(align_mask[b, :, qsl] == 0, np.float32(LOGM), np.float32(SHIFT))
        if use_mask:
            amk = attention_mask[b, 0, qsl, :].T  # [K, Q-slice]
            logm = np.maximum(logm + amk, np.float32(LOGM))
        logm = logm * np.float32(SCH_A / 32.0)
        # [K, Qh] -> [qb, p, t, q] with k = t*128 + p, qcol = qb*256 + q
        lm = logm.reshape(NKT, P, NQB, QBLK).transpose(2, 1, 0, 3)
        m = {
            "xqT": np.ascontiguousarray(q_hidden_states[b, qsl].T).astype(bf16),
            "xkT": np.ascontiguousarray(k_hidden_states[b].T).astype(bf16),
            "wqT": wq,
            "wkT": wk,
            "wvT": wv,
            "woT": wo,
            "logm8": np.ascontiguousarray(lm).astype(f8),
        }
        in_maps.append(m)

    global _last_in_maps
    _last_in_maps = in_maps
    res = run_bass_kernel_spmd(nc, in_maps, list(range(8))).results
    out = np.empty((B, Q, HID), np.float32)
    for core in range(8):
        b, qh = divmod(core, 2)
        out[b, qh * qh_len : (qh + 1) * qh_len] = res[core]["out"]
    return out


# revision 8
# speedup vs baseline: 1.0852x; 1.0189x over previous
"""KgAdapterCrossAttention kernel for 8 trn2 NeuronCores — v2.

Sharding: core = (batch b, query-half qh). Each core: 1024 queries x 2048 keys,
4 heads of 64 dims.

Techniques vs baseline (fp32 everywhere):
  - projections and A/O matmuls in bf16 (PE 1 cycle/row vs fp32's 4),
  - scores via error-compensated fp8e4m3 DoubleRow: with q = q_hi + q_lo and
    k = k_hi + k_lo (residual quantization), two DR matmuls per (head, ktile)
    compute K_hi'(q_hi + q_lo) and K_lo'q_hi + I*logM at 0.5 cycles/row —
    near-full-precision scores at half the bf16 PE cost, with the align-mask
    add fused into the last DR slot for free,
  - exp split across three engines: Act (native Exp -> bf16 P) plus DVE and
    Pool (Schraudolph fast-exp: int16 bits = S*184.665 + B, bitcast to bf16;
    the fused logM (-30) makes masked entries ~2^-40),
  - per-head softmax denominator from a ones-column appended to V,
  - O projection in bf16 after PE transposes.

Softmax has no max-subtraction; all logits carry a uniform -4 shift (softmax
invariant) so exp stays small; Schraudolph's constant factor cancels in the
p/sum(p) ratio.
"""

import os
import sys

import numpy as np

try:
    import concourse.bass as bass
except ImportError:
    for _p in ("/opt/trn_rl_repo", os.path.expanduser("~/.axon_site/_ro/trn_rl_repo")):
        if os.path.isdir(_p) and _p not in sys.path:
            sys.path.insert(0, _p)
    import concourse.bass as bass

import ml_dtypes
import concourse.mybir as mybir
import concourse.tile as tile
from concourse import bacc
from concourse.masks import make_identity
from contextlib import ExitStack

F32 = mybir.dt.float32
BF16 = mybir.dt.bfloat16
FP8 = mybir.dt.float8e4
I16 = mybir.dt.int16
FP16 = mybir.dt.float16
EXP = mybir.ActivationFunctionType.Exp
DR = mybir.MatmulPerfMode.DoubleRow
MUL = mybir.AluOpType.mult
ADD = mybir.AluOpType.add
SUB = mybir.AluOpType.subtract

P = 128
HID = 256
NHEAD = 4
DHEAD = 64
NQ = 1024  # queries per core
NK = 2048  # keys (full)
QBLK = 256
NQB = NQ // QBLK  # 4
NKT = NK // P  # 16
NCT = HID // P  # 2

# ktile assignment to exp engines: Act | DVE | Pool
NA = 11
ND = 5
NP = NKT - NA - ND  # 0 (gpsimd cannot access PSUM)

LOGM = -30.0   # masked-out additive bias (exact in e4m3); exp(-30+s) ~ 0
SHIFT = -4.0   # uniform logit shift (softmax-invariant): max score ~8.2 stays accurate
SCH_A = 184.664965  # 2^7 / ln 2
SCH_B = 16253.0     # 127*2^7 - 3 (Schraudolph bias; constant factor cancels)

# scr free-layout (fp8 elements per partition):
#   q region: (t, par) x (hi, lo) -> 8 x 1024, then logM double buffer 2 x 4096
SCR_Q = 8 * NQ            # 8192
SCR_W = SCR_Q + 2 * NKT * QBLK  # 16384
# kt8 free-layout per t-half: [ K_hi 2048 | K_lo 2048 | identity 128 ]
KT8_W = 2 * NK + P  # 4224


def _q_off(t: int, par: int, lo: int, qb: int) -> int:
    return ((2 * t + par) * 2 + lo) * NQ + qb * QBLK


def _m_off(qb: int, kt: int) -> int:
    return SCR_Q + (qb % 2) * NKT * QBLK + kt * QBLK


def build(with_attn_mask: bool) -> bass.Bass:
    del with_attn_mask  # attn mask folded into logm8 on host
    nc = bacc.Bacc()
    xqT = nc.declare_dram_parameter("xqT", [HID, NQ], BF16, isOutput=False)
    xkT = nc.declare_dram_parameter("xkT", [HID, NK], BF16, isOutput=False)
    wqT = nc.declare_dram_parameter("wqT", [HID, HID], BF16, isOutput=False)
    wkT = nc.declare_dram_parameter("wkT", [HID, HID], BF16, isOutput=False)
    wvT = nc.declare_dram_parameter("wvT", [HID, HID], BF16, isOutput=False)
    woT = nc.declare_dram_parameter("woT", [HID, HID], BF16, isOutput=False)
    logm8 = nc.declare_dram_parameter("logm8", [NQB, P, NKT, QBLK], FP8, isOutput=False)
    out_d = nc.declare_dram_parameter("out", [NQ, HID], F32, isOutput=True)

    with tile.TileContext(nc) as tc, ExitStack() as ctx:
        const = ctx.enter_context(tc.tile_pool(name="const", bufs=1))
        big = ctx.enter_context(tc.tile_pool(name="big", bufs=1))
        p16p = ctx.enter_context(tc.tile_pool(name="p16p", bufs=2))
        anp = ctx.enter_context(tc.tile_pool(name="anp", bufs=2))
        wrk = ctx.enter_context(tc.tile_pool(name="wrk", bufs=4))
        outp = ctx.enter_context(tc.tile_pool(name="outp", bufs=3))

        # --- weights + activations ---
        def load2(name, src, width):
            ts = []
            for t in range(2):
                tl = const.tile([P, width], BF16, tag=f"{name}{t}", name=f"{name}{t}")
                nc.sync.dma_start(out=tl, in_=src[t * P : (t + 1) * P, :])
                ts.append(tl)
            return ts

        wq_sb = load2("wq", wqT, HID)
        xq_sb = [big.tile([P, NQ], BF16, tag=f"xq{t}", name=f"xq{t}") for t in range(2)]
        xk_sb = [big.tile([P, NK], BF16, tag=f"xk{t}", name=f"xk{t}") for t in range(2)]
        for nb in range(NQ // 512):
            for t in range(2):
                nc.sync.dma_start(
                    out=xq_sb[t][:, nb * 512 : (nb + 1) * 512],
                    in_=xqT[t * P : (t + 1) * P, nb * 512 : (nb + 1) * 512],
                )
        wk_sb = load2("wk", wkT, HID)
        for nb in range(NK // 512):
            for t in range(2):
                nc.sync.dma_start(
                    out=xk_sb[t][:, nb * 512 : (nb + 1) * 512],
                    in_=xkT[t * P : (t + 1) * P, nb * 512 : (nb + 1) * 512],
                )
        wv_sb = load2("wv", wvT, HID)
        wo_sb = load2("wo", woT, HID)

        # fused scores scratch: q hi/lo halves (zero-padded) + logM double buffer
        scr = big.tile([P, SCR_W], FP8, tag="scr", name="scr")
        # kt8 per t-half: [K_hi | K_lo | identity] fp8
        kt8 = [big.tile([P, KT8_W], FP8, tag=f"kt8_{t}", name=f"kt8_{t}") for t in range(2)]
        for t in range(2):
            idap = kt8[t][:, 2 * NK : 2 * NK + P]
            nc.gpsimd.memset(idap, 0.0)
            nc.gpsimd.affine_select(
                out=idap, in_=idap, compare_op=mybir.AluOpType.not_equal,
                fill=32.0, base=0, pattern=[[-1, P]], channel_multiplier=1,
            )

        # zero the dead half-rows of q regions (even heads live in rows 0:64,
        # odd heads in rows 64:128; the other half must be 0 for the fused DR)
        for t in range(2):
            for lo in range(2):
                o0 = _q_off(t, 0, lo, 0)
                o1 = _q_off(t, 1, lo, 0)
                nc.gpsimd.memset(scr[DHEAD:P, o0 : o0 + NQ], 0.0)
                nc.gpsimd.memset(scr[0:DHEAD, o1 : o1 + NQ], 0.0)

        # V (bf16) with denominator ones-column
        v16 = big.tile([P, NKT, NHEAD, DHEAD + 1], BF16, tag="v16", name="v16")
        nc.gpsimd.memset(v16[:, :, :, DHEAD : DHEAD + 1], 1.0)

        with ExitStack() as pctx:
            ps_pr = pctx.enter_context(tc.tile_pool(name="ps_pr", bufs=8, space="PSUM"))
            # Q projection -> q_hi fp8 + q_lo fp8 (head-parity split, zero halves)
            for t in range(2):
                for nb in range(NQ // 512):
                    ps = ps_pr.tile([P, 512], F32, tag="pr", name="pr")
                    for ct in range(NCT):
                        nc.tensor.matmul(
                            ps,
                            lhsT=wq_sb[ct][:, t * P : (t + 1) * P],
                            rhs=xq_sb[ct][:, nb * 512 : (nb + 1) * 512],
                            start=(ct == 0),
                            stop=(ct == NCT - 1),
                        )
                    for par, rows in ((0, slice(0, DHEAD)), (1, slice(DHEAD, P))):
                        hi = _q_off(t, par, 0, 0) + nb * 512
                        lo = _q_off(t, par, 1, 0) + nb * 512
                        nc.scalar.copy(scr[rows, hi : hi + 512], ps[rows, :])
                        nc.vector.scalar_tensor_tensor(
                            scr[rows, lo : lo + 512],
                            ps[rows, :],
                            1.0,
                            scr[rows, hi : hi + 512],
                            op0=MUL,
                            op1=SUB,
                        )
            # K projection -> k_hi + k_lo fp8
            for t in range(2):
                for nb in range(NK // 512):
                    ps = ps_pr.tile([P, 512], F32, tag="pr", name="pr")
                    for ct in range(NCT):
                        nc.tensor.matmul(
                            ps,
                            lhsT=wk_sb[ct][:, t * P : (t + 1) * P],
                            rhs=xk_sb[ct][:, nb * 512 : (nb + 1) * 512],
                            start=(ct == 0),
                            stop=(ct == NCT - 1),
                        )
                    c0 = nb * 512
                    nc.scalar.copy(kt8[t][:, c0 : c0 + 512], ps)
                    nc.vector.scalar_tensor_tensor(
                        kt8[t][:, NK + c0 : NK + c0 + 512],
                        ps,
                        1.0,
                        kt8[t][:, c0 : c0 + 512],
                        op0=MUL,
                        op1=SUB,
                    )
            # V projection -> v16 bf16
            for kt in range(NKT):
                ps = ps_pr.tile([P, HID], F32, tag="pr", name="pr")
                for ct in range(NCT):
                    nc.tensor.matmul(
                        ps,
                        lhsT=xk_sb[ct][:, kt * P : (kt + 1) * P],
                        rhs=wv_sb[ct],
                        start=(ct == 0),
                        stop=(ct == NCT - 1),
                    )
                psr = ps.rearrange("p (h d) -> p h d", h=NHEAD)
                if kt % 2 == 0:
                    nc.vector.tensor_copy(v16[:, kt, :, 0:DHEAD], psr)
                else:
                    nc.scalar.copy(v16[:, kt, :, 0:DHEAD], psr)

        # --- attention ---
        with ExitStack() as actx:
            ps_sc = actx.enter_context(tc.tile_pool(name="ps_sc", bufs=3, space="PSUM"))
            ps_ao = actx.enter_context(tc.tile_pool(name="ps_ao", bufs=2, space="PSUM"))

            GROUPS = [list(range(g * 4, (g + 1) * 4)) for g in range(NKT // 4)]

            def seg_engine(kt):
                return 0 if kt < NA else (1 if kt < NA + ND else 2)

            def emit_score_group(qb, h, p16, grp, seg_fn=None):
                """PE: 2 fused DR matmuls per ktile; Act/DVE/Pool drain to P."""
                seg = seg_fn or seg_engine
                t, par = h // 2, h % 2
                qhi = _q_off(t, par, 0, qb)
                qlo = _q_off(t, par, 1, qb)
                ps = ps_sc.tile([P, 4, QBLK], F32, tag="sc", name="sc")
                for j, kt in enumerate(grp):
                    # DR1: K_hi' q_hi + K_hi' q_lo
                    lhs1 = bass.AP(
                        kt8[t].tensor,
                        kt8[t].offset + kt * P,
                        [[KT8_W, P], [0, 2], [1, P]],
                    )
                    rhs1 = bass.AP(
                        scr.tensor,
                        scr.offset + qhi,
                        [[SCR_W, P], [qlo - qhi, 2], [1, QBLK]],
                    )
                    nc.tensor.matmul(
                        ps[:, j, :], lhsT=lhs1, rhs=rhs1,
                        start=True, stop=False, perf_mode=DR,
                    )
                    # DR2: K_lo' q_hi + I * logM
                    lhs2 = bass.AP(
                        kt8[t].tensor,
                        kt8[t].offset + NK + kt * P,
                        [[KT8_W, P], [NK - kt * P, 2], [1, P]],
                    )
                    rhs2 = bass.AP(
                        scr.tensor,
                        scr.offset + qhi,
                        [[SCR_W, P], [_m_off(qb, kt) - qhi, 2], [1, QBLK]],
                    )
                    nc.tensor.matmul(
                        ps[:, j, :], lhsT=lhs2, rhs=rhs2,
                        start=False, stop=True, perf_mode=DR,
                    )
                # drain by engine segment (kt ranges are contiguous)
                j0 = 0
                ng = len(grp)
                while j0 < ng:
                    eng = seg(grp[j0])
                    j1 = j0
                    while j1 < ng and seg(grp[j1]) == eng:
                        j1 += 1
                    src = ps[:, j0:j1, :]
                    dst = p16[:, grp[j0] : grp[j1 - 1] + 1, :]
                    if eng == 0:
                        nc.scalar.activation(dst, src, EXP, scale=float(1.0 / SCH_A))
                    else:
                        nc.vector.tensor_scalar(
                            dst.bitcast(I16), src, SCH_B, None, ADD
                        )
                    j0 = j1

            def emit_a_qt(st, qt):
                """PE: A(qt) = P^T V (+denom col); DVE recip; Pool normalize."""
                qb, h, p16, anorm, ap_t = st
                ap_ = ap_t[:, qt, :]
                qsl = slice(qt * P, (qt + 1) * P)
                for kt in range(NKT):
                    nc.tensor.matmul(
                        ap_,
                        lhsT=p16[:, kt, qsl],
                        rhs=v16[:, kt, h, :],
                        start=(kt == 0),
                        stop=(kt == NKT - 1),
                    )
                rec = wrk.tile([P, 1], F32, tag="rec", name="rec")
                nc.vector.reciprocal(rec, ap_[:, DHEAD : DHEAD + 1])
                nc.vector.tensor_scalar(
                    anorm[:, qt, h * DHEAD : (h + 1) * DHEAD],
                    ap_[:, 0:DHEAD],
                    rec,
                    None,
                    MUL,
                )

            def emit_oproj_qt(qb, anorm, qt):
                # anorm^T via DMA transpose (no PE/DVE cost), then bf16 matmul;
                # o accum shares the ps_ao union bank with the A tile
                bank = ps_ao.tile([P, 512], F32, tag="ao", name="ao")
                o_ps = bank[:, 0:256]
                atts = []
                for ct in range(NCT):
                    att = wrk.tile([P, P], BF16, tag=f"att{ct}", name=f"att{ct}")
                    nc.sync.dma_start_transpose(
                        att, anorm[:, qt, ct * P : (ct + 1) * P]
                    )
                    atts.append(att)
                for ct in range(NCT):
                    nc.tensor.matmul(
                        o_ps, lhsT=atts[ct], rhs=wo_sb[ct],
                        start=(ct == 0), stop=(ct == NCT - 1),
                    )
                ob = outp.tile([P, HID], F32, tag="ob", name="ob")
                nc.vector.tensor_copy(ob, o_ps)
                q0 = qb * QBLK + qt * P
                nc.sync.dma_start(out=out_d[q0 : q0 + P, :], in_=ob)

            # Software pipeline: A(i) (two qt halves) and O-proj quarters are
            # interleaved between score groups of step i+1 so PE never waits
            # on exp drains and the clock ramp is preserved.
            steps = [(qb, h) for qb in range(NQB) for h in range(NHEAD)]
            pend_a = None  # (qb, h, p16, anorm, ap_t)
            pend_o = []    # [(qb, anorm, qt), ...]
            anorms = {}
            def fetch_logm(qb):
                mb = _m_off(qb, 0)
                nc.sync.dma_start(
                    out=scr[:, mb : mb + NKT * QBLK],
                    in_=logm8[qb].rearrange("p t q -> p (t q)"),
                )

            fetch_logm(0)
            for qb, h in steps:
                if h == 0:
                    anorms[qb] = anp.tile([P, 2, HID], BF16, tag="an", name="an")
                if h == 1 and qb + 1 < NQB:
                    fetch_logm(qb + 1)
                p16 = p16p.tile([P, NKT, QBLK], BF16, tag="p16", name="p16")
                last = qb == NQB - 1 and h == NHEAD - 1
                seg_fn = (lambda kt: (0, 1, 0, 1, 0, 1, 0, 1, 0, 1, 0, 1, 0, 1, 0, 1)[kt]) if last else None
                emit_score_group(qb, h, p16, GROUPS[0], seg_fn)
                emit_score_group(qb, h, p16, GROUPS[1], seg_fn)
                if pend_a is not None:
                    emit_a_qt(pend_a, 0)
                emit_score_group(qb, h, p16, GROUPS[2], seg_fn)
                if pend_a is not None:
                    emit_a_qt(pend_a, 1)
                emit_score_group(qb, h, p16, GROUPS[3], seg_fn)
                if pend_o:
                    emit_oproj_qt(*pend_o.pop(0))
                abank = ps_ao.tile([P, 512], F32, tag="ao", name="ao")
                ap_t = abank[:, 256 : 256 + 2 * (DHEAD + 1)].rearrange(
                    "p (a b) -> p a b", a=2
                )
                pend_a = (qb, h, p16, anorms[qb], ap_t)
                if h == 1 and qb > 0:
                    an = anorms.pop(qb - 1)
                    pend_o += [(qb - 1, an, 0), (qb - 1, an, 1)]
                if last:
                    # drain tail immediately, A interleaved per group already
                    emit_a_qt(pend_a, 0)
                    emit_a_qt(pend_a, 1)
                    pend_a = None
                    an = anorms.pop(NQB - 1)
                    emit_oproj_qt(NQB - 1, an, 0)
                    emit_oproj_qt(NQB - 1, an, 1)
    nc.compile()
    return nc


_NC_CACHE = {}
_last_in_maps = None


def _get_nc(with_attn_mask: bool = False) -> bass.Bass:
    if with_attn_mask not in _NC_CACHE:
        _NC_CACHE[with_attn_mask] = build(with_attn_mask)
    return _NC_CACHE[with_attn_mask]


def kernel(q_hidden_states, k_hidden_states, attention_mask, align_mask, Wq, Wk, Wv, Wo):
    from concourse.bass_utils import run_bass_kernel_spmd

    q_hidden_states = np.asarray(q_hidden_states, np.float32)
    k_hidden_states = np.asarray(k_hidden_states, np.float32)
    attention_mask = np.asarray(attention_mask, np.float32)
    align_mask = np.asarray(align_mask)
    B, Q, _ = q_hidden_states.shape
    qh_len = Q // 2  # 1024

    use_mask = bool(np.any(attention_mask))
    nc = _get_nc(False)

    bf16 = ml_dtypes.bfloat16
    f8 = ml_dtypes.float8_e4m3
    s8 = np.float32(np.sqrt(SCH_A) / np.sqrt(8.0))
    wq = np.ascontiguousarray(np.asarray(Wq, np.float32).T * s8).astype(bf16)
    wk = np.ascontiguousarray(np.asarray(Wk, np.float32).T * s8).astype(bf16)
    wv = np.ascontiguousarray(np.asarray(Wv, np.float32).T).astype(bf16)
    wo = np.ascontiguousarray(np.asarray(Wo, np.float32).T).astype(bf16)

    in_maps = []
    for core in range(8):
        b, qh = divmod(core, 2)
        qsl = slice(qh * qh_len, (qh + 1) * qh_len)
        logm = np.where(align_mask[b, :, qsl] == 0, np.float32(LOGM), np.float32(SHIFT))
        if use_mask:
            amk = attention_mask[b, 0, qsl, :].T  # [K, Q-slice]
            logm = np.maximum(logm + amk, np.float32(LOGM))
        logm = logm * np.float32(SCH_A / 32.0)
        # [K, Qh] -> [qb, p, t, q] with k = t*128 + p, qcol = qb*256 + q
        lm = logm.reshape(NKT, P, NQB, QBLK).transpose(2, 1, 0, 3)
        m = {
            "xqT": np.ascontiguousarray(q_hidden_states[b, qsl].T).astype(bf16),
            "xkT": np.ascontiguousarray(k_hidden_states[b].T).astype(bf16),
            "wqT": wq,
            "wkT": wk,
            "wvT": wv,
            "woT": wo,
            "logm8": np.ascontiguousarray(lm).astype(f8),
        }
        in_maps.append(m)

    global _last_in_maps
    _last_in_maps = in_maps
    res = run_bass_kernel_spmd(nc, in_maps, list(range(8))).results
    out = np.empty((B, Q, HID), np.float32)
    for core in range(8):
        b, qh = divmod(core, 2)
        out[b, qh * qh_len : (qh + 1) * qh_len] = res[core]["out"]
    return out
